# revision 20
# baseline (speedup 1.0000x reference)
"""Trainium2 Bass kernel for DigitConvolutionalModel.

Math: the 3x3 valid conv is a linear map, so it folds into the first Linear
layer on the host (O(1) w.r.t. batch):  out = relu(x @ W_eff + b1) @ w2.T + b2
with W_eff[784, 128].  Distribution is pure data parallel: batch sharded
across 8 NeuronCores, weights replicated, each core computing [10, 8192].

dtypes: x ships as fp8 e3m4 (4 mantissa bits; subnormals handled exactly by
the PE's FP22 upconvert) against fp16 weights — this halves HBM traffic vs
fp16 and costs ~1.3e-2 max rel error on this data (gate: 2e-2).  fp32 PSUM
accumulation; h is emitted fp16 for the second matmul.

Two PE-array packing tricks remove the under-utilized matmul passes:

- The K=16 remainder matmul wastes 112/128 PE rows.  Batch tiles group
  in quads: tile 4q+j's remainder runs in PE row-strip 32j via
  tile_position=(32j, 0).  Row-disjoint matmuls execute concurrently
  (Dstart ~4ns), so 4 remainder passes cost ~1 pass.
- The M=10 second matmul wastes 118/128 PE columns.  Per quad, the 4 mm2s
  run in column strips via tile_position=(0, 32j), writing partition strip
  32j..32j+9 of one shared PSUM bank.  4 passes cost ~1.

Per quad: 24 full mm1 passes + 1 remainder burst + 1 mm2 burst = 26 passes
vs 32 unpacked.

The quad epilogue pipeline: quad q's mm2 burst is emitted after quad q+1's
mm1 chains, so the PE never waits on the DVE relu chain.  b2 is replicated
per row-strip so one DVE op biases all 4 tiles; each quad stores its whole
[128, 512] strip block in one DMA (rows 10..31 of each strip are don't-care)
and the host extracts the 10 valid rows per strip.
"""

import numpy as np
import ml_dtypes

import concourse.bass as bass  # noqa: F401  (bass registers mybir lowerings)
import concourse.mybir as mybir
import concourse.tile as tile
from concourse import bacc
from concourse.bass_utils import run_bass_kernel_spmd

N_CORES = 8
B = 65536
B_SH = B // N_CORES  # 8192 rows per core
D = 784              # 28*28 input features
DM = 768             # features in the main 128-partition stream
DR = D - DM          # 16 remainder features
H = 128              # hidden
OUT = 10
KT = 128             # contraction tile = full partition dim
NK = DM // KT        # 6 main K-tiles
NB = 512             # batch columns per tile (= one fp32 PSUM bank)
NT = B_SH // NB      # 16 batch tiles
NQ = NT // 4         # quads of batch tiles

# one x DMA per batch tile, alternating rings: each ring's deadline
# profile then tracks half the PE consumption rate, which is what a
# single HWDGE ring can actually sustain (~150 GB/s when both are busy)
GROUPS = [(t, t + 1) for t in range(NT)]
N_WARM = 12  # bridge PE from engine-start (~7.4us) to t0 arrival (~11.7us)
             # with no idle gap, so the HAM clock never re-throttles

_CACHE = {}


def _build_nc():
    f32 = mybir.dt.float32
    f16 = mybir.dt.float16
    f8 = mybir.dt.float8e3
    nc = bacc.Bacc("TRN2", target_bir_lowering=False, debug=False,
                   num_devices=N_CORES)
    xtp = nc.dram_tensor("xtp", [KT, NT, NK, NB], f8,
                         kind="ExternalInput").ap()
    # remainder features per row-strip: [32j+r, q, c] = feature 768+r of
    # batch tile 4q+j (r<16; rows 16..31 of each strip are zero padding)
    xr4 = nc.dram_tensor("xr4", [KT, NQ, NB], f8, kind="ExternalInput").ap()
    wm = nc.dram_tensor("wm", [KT, NK, H], f16, kind="ExternalInput").ap()
    # remainder weights replicated into each row-strip
    wr4 = nc.dram_tensor("wr4", [KT, H], f16, kind="ExternalInput").ap()
    w2t = nc.dram_tensor("w2t", [H, OUT], f16, kind="ExternalInput").ap()
    # biasd[:, 0] = b1; biasd[32j+r, 1] = b2[r] (r<10)
    biasd = nc.dram_tensor("biasd", [KT, 2], f32, kind="ExternalInput").ap()
    # out4[32j+r, q, c] = logit r of batch row (4q+j)*512+c (r<10; rows
    # 10..31 of each strip are don't-care) — one store per quad keeps the
    # tail short (each store trigger costs ~0.75us of engine time)
    out4 = nc.dram_tensor("out4", [KT, NQ, NB], f32,
                          kind="ExternalOutput").ap()

    with tile.TileContext(nc) as tc:
        with (
            tc.tile_pool(name="wpool", bufs=1) as wpool,
            tc.tile_pool(name="xpool", bufs=1) as xpool,
            tc.tile_pool(name="hpool", bufs=8) as hpool,
            tc.tile_pool(name="opool", bufs=2) as opool,
            tc.tile_pool(name="ps1", bufs=4, space="PSUM") as ps1pool,
            tc.tile_pool(name="ps2", bufs=2, space="PSUM") as ps2pool,
        ):
            # PE warm consumption (~293 GB/s of fp8 x) roughly equals the
            # two HWDGE rings' combined supply (~300 GB/s), so the
            # schedule's job is ordering: wm leads the sync ring (the
            # first chain blocks on it), each ring then streams its x
            # tiles back-to-back, the tiny second-layer weights slot in
            # behind scalar's first tile, and xr4 rides the (slow-start
            # but otherwise idle) gpsimd SWDGE queue — it isn't needed
            # until the first remainder burst at ~17us.
            x_sb = xpool.tile([KT, NT, NK, NB], f8)
            w_sb = wpool.tile([KT, NK, H], f16)
            w2_sb = wpool.tile([H, OUT], f16)
            bias_sb = wpool.tile([KT, 2], f32)
            wr_sb = wpool.tile([KT, H], f16)
            xr_sb = wpool.tile([KT, NQ, NB], f8)
            def xg(gi):
                a, b = GROUPS[gi]
                eng = (nc.sync, nc.scalar)[gi % 2]
                eng.dma_start(x_sb[:, a:b, :, :], xtp[:, a:b, :, :])

            # only the 32KB k=0 weight slice must precede t0 — the k>=1
            # slices are first touched 216ns+ into the chain, so they ride
            # behind t0 and overlap the k=0 matmul
            nc.sync.dma_start(w_sb[:, 0:1, :], wm[:, 0:1, :])
            xg(0)
            nc.sync.dma_start(w_sb[:, 1:NK, :], wm[:, 1:NK, :])
            xg(1)
            nc.scalar.dma_start(wr_sb[:], wr4[:])
            nc.scalar.dma_start(w2_sb[:], w2t[:])
            nc.scalar.dma_start(bias_sb[:], biasd[:])
            nc.gpsimd.dma_start(xr_sb[:], xr4[:])
            for gi in range(2, len(GROUPS)):
                xg(gi)

            warm_x = wpool.tile([KT, NB], f16)
            nc.vector.memset(warm_x[:], 0.0)
            warm_ps = ps1pool.tile([H, NB], f32, tag="ps1")
            for _ in range(N_WARM):
                nc.tensor.matmul(warm_ps[:], lhsT=warm_x[:, 0:H],
                                 rhs=warm_x[:], start=True, stop=True)

            def mm2_store_burst(q, hs):
                # 4 col-tiled mm2 passes into one shared PSUM bank
                ps2 = ps2pool.tile([KT, NB], f32, name="ps2")
                for j in range(4):
                    nc.tensor.matmul(
                        ps2[32 * j:32 * j + OUT, :],
                        lhsT=w2_sb[:], rhs=hs[j][:],
                        start=True, stop=True,
                        tile_position=(0, 32 * j),
                    )
                o_sb = opool.tile([KT, NB], f32, name="o_sb")
                nc.vector.tensor_scalar_add(o_sb[:], ps2[:], bias_sb[:, 1:2])
                if q == NQ - 1:
                    # tail store is on the critical path: split it across
                    # both (by now idle) rings so the halves move in parallel
                    nc.sync.dma_start(out4[0:64, q, :], o_sb[0:64, :])
                    nc.scalar.dma_start(out4[64:KT, q, :], o_sb[64:KT, :])
                else:
                    eng = (nc.sync, nc.scalar)[q % 2]
                    eng.dma_start(out4[:, q, :], o_sb[:])

            prev = None
            for q in range(NQ):
                ps1s = []
                for j in range(4):
                    t = 4 * q + j
                    ps1 = ps1pool.tile([H, NB], f32, name="ps1")
                    for k in range(NK):
                        nc.tensor.matmul(
                            ps1[:],
                            lhsT=w_sb[:, k, :],
                            rhs=x_sb[:, t, k, :],
                            start=(k == 0),
                            stop=False,
                        )
                    ps1s.append(ps1)
                # remainder burst: 4 row-tiled K=16 passes, one per strip
                for j in range(4):
                    nc.tensor.matmul(
                        ps1s[j][:],
                        lhsT=wr_sb[32 * j:32 * j + DR, :],
                        rhs=xr_sb[32 * j:32 * j + DR, q, :],
                        start=False, stop=True,
                        tile_position=(32 * j, 0),
                    )
                if prev is not None:
                    mm2_store_burst(*prev)
                hs = []
                for j in range(4):
                    h_sb = hpool.tile([H, NB], f16, name="h_sb")
                    nc.vector.tensor_scalar(
                        h_sb[:], ps1s[j][:], bias_sb[:, 0:1], 0.0,
                        mybir.AluOpType.add, mybir.AluOpType.max)
                    hs.append(h_sb)
                prev = (q, hs)
            mm2_store_burst(*prev)

    nc.compile()
    return nc


def _get_nc():
    if "nc" not in _CACHE:
        _CACHE["nc"] = _build_nc()
    return _CACHE["nc"]


def _fold_weights(conv_w: np.ndarray, w1: np.ndarray) -> np.ndarray:
    """W_eff[784, 128]: h_pre = x @ W_eff  ==  conv(x) @ w1.T  (float64 accum)."""
    w1k = w1.reshape(H, 26, 26).transpose(1, 2, 0).astype(np.float64)  # [i,j,k]
    cw = conv_w.astype(np.float64)
    W = np.zeros((28, 28, H), np.float64)
    for di in range(3):
        for dj in range(3):
            W[di:di + 26, dj:dj + 26, :] += cw[di, dj] * w1k
    return W.reshape(D, H).astype(np.float32)


def make_in_maps(x, conv_w, w1, b1, w2, b2):
    x = np.asarray(x, np.float32)
    weff = _fold_weights(np.asarray(conv_w, np.float32),
                         np.asarray(w1, np.float32))
    wm = np.ascontiguousarray(
        weff[:DM].reshape(NK, KT, H).transpose(1, 0, 2)).astype(np.float16)
    wr4 = np.zeros((KT, H), np.float16)
    for j in range(4):
        wr4[32 * j:32 * j + DR] = weff[DM:].astype(np.float16)
    w2t = np.ascontiguousarray(np.asarray(w2, np.float32).T).astype(np.float16)
    biasd = np.zeros((KT, 2), np.float32)
    biasd[:, 0] = np.asarray(b1, np.float32)
    for j in range(4):
        biasd[32 * j:32 * j + OUT, 1] = np.asarray(b2, np.float32)
    in_maps = []
    for i in range(N_CORES):
        xq = x[i * B_SH:(i + 1) * B_SH].astype(ml_dtypes.float8_e3m4)
        xtp = xq[:, :DM].reshape(NT, NB, NK, KT).transpose(3, 0, 2, 1)
        # [q, j, c, r] -> strip layout [4j, r, q, c] padded to 32 rows/strip
        r16 = xq[:, DM:].reshape(NQ, 4, NB, DR).transpose(1, 3, 0, 2)
        xr4 = np.zeros((4, 32, NQ, NB), ml_dtypes.float8_e3m4)
        xr4[:, :DR] = r16
        in_maps.append({"xtp": np.ascontiguousarray(xtp),
                        "xr4": np.ascontiguousarray(xr4.reshape(KT, NQ, NB)),
                        "wm": wm, "wr4": wr4, "w2t": w2t, "biasd": biasd})
    return in_maps


def kernel(x, conv_w, w1, b1, w2, b2):
    nc = _get_nc()
    in_maps = make_in_maps(x, conv_w, w1, b1, w2, b2)
    res = run_bass_kernel_spmd(nc, in_maps, list(range(N_CORES)))
    # out4[32j+r, q, c] -> out[(4q+j)*512+c, r]
    outs = []
    for i in range(N_CORES):
        o4 = res.results[i]["out4"].reshape(4, 32, NQ, NB)[:, :OUT]
        outs.append(o4.transpose(1, 2, 0, 3).reshape(OUT, B_SH))
    out = np.concatenate(outs, axis=1)
    return np.ascontiguousarray(out.T)  # [65536, 10] float32


# revision 21
# speedup vs baseline: 1.0719x; 1.0719x over previous
"""Trainium2 Bass kernel for DigitConvolutionalModel.

Math: the 3x3 valid conv is a linear map, so it folds into the first Linear
layer on the host (O(1) w.r.t. batch):  out = relu(x @ W_eff + b1) @ w2.T + b2
with W_eff[784, 128].  Distribution is pure data parallel: batch sharded
across 8 NeuronCores, weights replicated, each core computing [10, 8192].

dtypes: x ships as fp8 e3m4 (4 mantissa bits; subnormals handled exactly by
the PE's FP22 upconvert) against fp16 weights — this halves HBM traffic vs
fp16 and costs ~1.3e-2 max rel error on this data (gate: 2e-2).  fp32 PSUM
accumulation; h is emitted fp16 for the second matmul.

Two PE-array packing tricks remove the under-utilized matmul passes:

- The K=16 remainder matmul wastes 112/128 PE rows.  Batch tiles group
  in quads: tile 4q+j's remainder runs in PE row-strip 32j via
  tile_position=(32j, 0).  Row-disjoint matmuls execute concurrently
  (Dstart ~4ns), so 4 remainder passes cost ~1 pass.
- The M=10 second matmul wastes 118/128 PE columns.  Per quad, the 4 mm2s
  run in column strips via tile_position=(0, 32j), writing partition strip
  32j..32j+9 of one shared PSUM bank.  4 passes cost ~1.

Per quad: 24 full mm1 passes + 1 remainder burst + 1 mm2 burst = 26 passes
vs 32 unpacked.

The quad epilogue pipeline: quad q's mm2 burst is emitted after quad q+1's
mm1 chains, so the PE never waits on the DVE relu chain.  b2 is replicated
per row-strip so one DVE op biases all 4 tiles; each quad stores its whole
[128, 512] strip block in one DMA (rows 10..31 of each strip are don't-care)
and the host extracts the 10 valid rows per strip.
"""

import numpy as np
import ml_dtypes

import concourse.bass as bass  # noqa: F401  (bass registers mybir lowerings)
import concourse.mybir as mybir
import concourse.tile as tile
from concourse import bacc
from concourse.bass_utils import run_bass_kernel_spmd

N_CORES = 8
B = 65536
B_SH = B // N_CORES  # 8192 rows per core
D = 784              # 28*28 input features
DM = 768             # features in the main 128-partition stream
DR = D - DM          # 16 remainder features
H = 128              # hidden
OUT = 10
KT = 128             # contraction tile = full partition dim
NK = DM // KT        # 6 main K-tiles
NB = 512             # batch columns per tile (= one fp32 PSUM bank)
NT = B_SH // NB      # 16 batch tiles
NQ = NT // 4         # quads of batch tiles

# one x DMA per batch tile, alternating rings: each ring's deadline
# profile then tracks half the PE consumption rate, which is what a
# single HWDGE ring can actually sustain (~150 GB/s when both are busy)
GROUPS = [(t, t + 1) for t in range(NT)]
N_WARM = 21  # bridge PE from engine-start (~7.4us) to t0 arrival (~13.7us)
             # with no idle gap, so the HAM clock never re-throttles

_CACHE = {}


def _build_nc():
    f32 = mybir.dt.float32
    f16 = mybir.dt.float16
    f8 = mybir.dt.float8e3
    nc = bacc.Bacc("TRN2", target_bir_lowering=False, debug=False,
                   num_devices=N_CORES)
    xtp = nc.dram_tensor("xtp", [KT, NT, NK, NB], f8,
                         kind="ExternalInput").ap()
    # remainder features per row-strip: [32j+r, q, c] = feature 768+r of
    # batch tile 4q+j (r<16; rows 16..31 of each strip are zero padding)
    xr4 = nc.dram_tensor("xr4", [KT, NQ, NB], f8, kind="ExternalInput").ap()
    wm = nc.dram_tensor("wm", [KT, NK, H], f16, kind="ExternalInput").ap()
    # remainder weights replicated into each row-strip
    wr4 = nc.dram_tensor("wr4", [KT, H], f16, kind="ExternalInput").ap()
    w2t = nc.dram_tensor("w2t", [H, OUT], f16, kind="ExternalInput").ap()
    # biasd[:, 0] = b1; biasd[32j+r, 1] = b2[r] (r<10)
    biasd = nc.dram_tensor("biasd", [KT, 2], f32, kind="ExternalInput").ap()
    # out4[32j+r, q, c] = logit r of batch row (4q+j)*512+c (r<10; rows
    # 10..31 of each strip are don't-care) — one store per quad keeps the
    # tail short (each store trigger costs ~0.75us of engine time)
    out4 = nc.dram_tensor("out4", [KT, NQ, NB], f32,
                          kind="ExternalOutput").ap()

    with tile.TileContext(nc) as tc:
        with (
            tc.tile_pool(name="wpool", bufs=1) as wpool,
            tc.tile_pool(name="xpool", bufs=1) as xpool,
            tc.tile_pool(name="hpool", bufs=8) as hpool,
            tc.tile_pool(name="opool", bufs=2) as opool,
            tc.tile_pool(name="ps1", bufs=4, space="PSUM") as ps1pool,
            tc.tile_pool(name="ps2", bufs=2, space="PSUM") as ps2pool,
        ):
            # PE warm consumption (~293 GB/s of fp8 x) roughly equals the
            # two HWDGE rings' combined supply (~300 GB/s), so the
            # schedule's job is ordering: wm leads the sync ring (the
            # first chain blocks on it), each ring then streams its x
            # tiles back-to-back, the tiny second-layer weights slot in
            # behind scalar's first tile, and xr4 rides the (slow-start
            # but otherwise idle) gpsimd SWDGE queue — it isn't needed
            # until the first remainder burst at ~17us.
            x_sb = xpool.tile([KT, NT, NK, NB], f8)
            w_sb = wpool.tile([KT, NK, H], f16)
            w2_sb = wpool.tile([H, OUT], f16)
            bias_sb = wpool.tile([KT, 2], f32)
            wr_sb = wpool.tile([KT, H], f16)
            xr_sb = wpool.tile([KT, NQ, NB], f8)
            def xg(gi):
                a, b = GROUPS[gi]
                eng = (nc.sync, nc.scalar)[gi % 2]
                eng.dma_start(x_sb[:, a:b, :, :], xtp[:, a:b, :, :])

            nc.sync.dma_start(w_sb[:], wm[:])
            xg(0)
            xg(1)
            nc.scalar.dma_start(wr_sb[:], wr4[:])
            nc.scalar.dma_start(w2_sb[:], w2t[:])
            nc.scalar.dma_start(bias_sb[:], biasd[:])
            nc.gpsimd.dma_start(xr_sb[:], xr4[:])
            for gi in range(2, len(GROUPS)):
                xg(gi)

            warm_x = wpool.tile([KT, NB], f16)
            nc.vector.memset(warm_x[:], 0.0)
            warm_ps = ps1pool.tile([H, NB], f32, tag="ps1")
            for _ in range(N_WARM):
                nc.tensor.matmul(warm_ps[:], lhsT=warm_x[:, 0:H],
                                 rhs=warm_x[:], start=True, stop=True)

            def mm2_store_burst(q, hs):
                # 4 col-tiled mm2 passes into one shared PSUM bank
                ps2 = ps2pool.tile([KT, NB], f32, name="ps2")
                for j in range(4):
                    nc.tensor.matmul(
                        ps2[32 * j:32 * j + OUT, :],
                        lhsT=w2_sb[:], rhs=hs[j][:],
                        start=True, stop=True,
                        tile_position=(0, 32 * j),
                    )
                o_sb = opool.tile([KT, NB], f32, name="o_sb")
                nc.vector.tensor_scalar_add(o_sb[:], ps2[:], bias_sb[:, 1:2])
                eng = (nc.sync, nc.scalar)[q % 2]
                eng.dma_start(out4[:, q, :], o_sb[:])

            prev = None
            for q in range(NQ):
                ps1s = []
                for j in range(4):
                    t = 4 * q + j
                    ps1 = ps1pool.tile([H, NB], f32, name="ps1")
                    for k in range(NK):
                        nc.tensor.matmul(
                            ps1[:],
                            lhsT=w_sb[:, k, :],
                            rhs=x_sb[:, t, k, :],
                            start=(k == 0),
                            stop=False,
                        )
                    ps1s.append(ps1)
                # remainder burst: 4 row-tiled K=16 passes, one per strip
                for j in range(4):
                    nc.tensor.matmul(
                        ps1s[j][:],
                        lhsT=wr_sb[32 * j:32 * j + DR, :],
                        rhs=xr_sb[32 * j:32 * j + DR, q, :],
                        start=False, stop=True,
                        tile_position=(32 * j, 0),
                    )
                if prev is not None:
                    mm2_store_burst(*prev)
                hs = []
                for j in range(4):
                    h_sb = hpool.tile([H, NB], f16, name="h_sb")
                    nc.vector.tensor_scalar(
                        h_sb[:], ps1s[j][:], bias_sb[:, 0:1], 0.0,
                        mybir.AluOpType.add, mybir.AluOpType.max)
                    hs.append(h_sb)
                prev = (q, hs)
            mm2_store_burst(*prev)

    nc.compile()
    return nc


def _get_nc():
    if "nc" not in _CACHE:
        _CACHE["nc"] = _build_nc()
    return _CACHE["nc"]


def _fold_weights(conv_w: np.ndarray, w1: np.ndarray) -> np.ndarray:
    """W_eff[784, 128]: h_pre = x @ W_eff  ==  conv(x) @ w1.T  (float64 accum)."""
    w1k = w1.reshape(H, 26, 26).transpose(1, 2, 0).astype(np.float64)  # [i,j,k]
    cw = conv_w.astype(np.float64)
    W = np.zeros((28, 28, H), np.float64)
    for di in range(3):
        for dj in range(3):
            W[di:di + 26, dj:dj + 26, :] += cw[di, dj] * w1k
    return W.reshape(D, H).astype(np.float32)


def make_in_maps(x, conv_w, w1, b1, w2, b2):
    x = np.asarray(x, np.float32)
    weff = _fold_weights(np.asarray(conv_w, np.float32),
                         np.asarray(w1, np.float32))
    wm = np.ascontiguousarray(
        weff[:DM].reshape(NK, KT, H).transpose(1, 0, 2)).astype(np.float16)
    wr4 = np.zeros((KT, H), np.float16)
    for j in range(4):
        wr4[32 * j:32 * j + DR] = weff[DM:].astype(np.float16)
    w2t = np.ascontiguousarray(np.asarray(w2, np.float32).T).astype(np.float16)
    biasd = np.zeros((KT, 2), np.float32)
    biasd[:, 0] = np.asarray(b1, np.float32)
    for j in range(4):
        biasd[32 * j:32 * j + OUT, 1] = np.asarray(b2, np.float32)
    in_maps = []
    for i in range(N_CORES):
        xq = x[i * B_SH:(i + 1) * B_SH].astype(ml_dtypes.float8_e3m4)
        xtp = xq[:, :DM].reshape(NT, NB, NK, KT).transpose(3, 0, 2, 1)
        # [q, j, c, r] -> strip layout [4j, r, q, c] padded to 32 rows/strip
        r16 = xq[:, DM:].reshape(NQ, 4, NB, DR).transpose(1, 3, 0, 2)
        xr4 = np.zeros((4, 32, NQ, NB), ml_dtypes.float8_e3m4)
        xr4[:, :DR] = r16
        in_maps.append({"xtp": np.ascontiguousarray(xtp),
                        "xr4": np.ascontiguousarray(xr4.reshape(KT, NQ, NB)),
                        "wm": wm, "wr4": wr4, "w2t": w2t, "biasd": biasd})
    return in_maps


def kernel(x, conv_w, w1, b1, w2, b2):
    nc = _get_nc()
    in_maps = make_in_maps(x, conv_w, w1, b1, w2, b2)
    res = run_bass_kernel_spmd(nc, in_maps, list(range(N_CORES)))
    # out4[32j+r, q, c] -> out[(4q+j)*512+c, r]
    outs = []
    for i in range(N_CORES):
        o4 = res.results[i]["out4"].reshape(4, 32, NQ, NB)[:, :OUT]
        outs.append(o4.transpose(1, 2, 0, 3).reshape(OUT, B_SH))
    out = np.concatenate(outs, axis=1)
    return np.ascontiguousarray(out.T)  # [65536, 10] float32
